# revision 38
# baseline (speedup 1.0000x reference)
"""Trainium2 Bass kernel for nn_MISA (dense_transformer, data-parallel over 8 cores).

Layout: feature-major activations [feat_part=128, mtile, batch_cols] per core.
Batch 4096 -> 512 per core -> two passes of 256 columns.
All matmuls bf16 (fp32 PSUM accumulation); LN/softmax internals fp32.

v2 (DMA restructure): weights live in DRAM as [128, K/128, M] and each
weight tile is DMA'd exactly once per pass:
- projS shares one stationary weight load across the 4 expand positions
  (pairs of positions ride one matmul: moving [128,2,N] -> PSUM [128,512]).
- PSUM evictions run on the Scalar engine (Identity+bias) to unload DVE.
- self-attn residual is folded into the out-proj PSUM via an identity matmul,
  and the out-proj evicts in place over the expand tile.
- output is written bf16 and cast to f32 on host.

Structural simplifications (exact, not approximations):
- attention with all-equal keys/values (q/k/v = broadcast joint row) is the
  identity on v: cross_tj == cross_sj == out_proj4(v_proj4(joint)).
- mean over query positions commutes with out_proj and with A@V, so the six
  cross outputs never materialize per-query outputs (abar-weighted V only).
- all-equal queries (j as q): single query row, output equals its mean.
"""
import sys, math
from contextlib import ExitStack
sys.path.insert(0, "/opt/trn_rl_repo")

import numpy as np
import ml_dtypes

import concourse.bass as bass
import concourse.mybir as mybir
from concourse import bacc
import concourse.tile as tile
from concourse import bass_utils

F32 = mybir.dt.float32
BF16 = mybir.dt.bfloat16
AF = mybir.ActivationFunctionType
ALU = mybir.AluOpType
BF = ml_dtypes.bfloat16

H = 8
E = 4
HD = 1024
B = 4096
NCORES = 8
BC = B // NCORES          # 512 batch per core
NP = 2                    # passes per core
N = BC // NP              # 256 batch cols per pass
EPS = 1e-5


def _bias_cols(b):
    # [M] -> [128, M//128]: column m = per-partition bias of m-tile m
    return np.ascontiguousarray(np.asarray(b, np.float32).reshape(-1, 128).T)


def build(res_w: float):
    nc = bacc.Bacc("TRN2", target_bir_lowering=False, debug=False)

    def din(name, shape, dt):
        return nc.dram_tensor(name, list(shape), dt, kind="ExternalInput").ap()

    xt_d = din("xt", (128, 8, BC), F32)
    xs_d = din("xs", (128, 8, BC), F32)
    # weights pair-blocked: [128, M/256, K/128, 256] — one (2-mtile, all-kt)
    # block is contiguous per partition, so block DMAs run at full rate
    wexp = [din(f"wexp{j}", (128, 16, 8, 256), BF16) for j in range(2)]
    bexp = [din(f"bexp{j}", (128, 32), F32) for j in range(2)]
    wqkv = [din(f"wqkv{i}", (128, 12, 8, 256), BF16) for i in range(5)]
    bqkv = [din(f"bqkv{i}", (128, 24), F32) for i in range(5)]
    wout = [din(f"wout{i}", (128, 4, 8, 256), BF16) for i in range(5)]
    bout = [din(f"bout{i}", (128, 8), F32) for i in range(5)]
    wjoint = din("wjoint", (128, 4, 16, 256), BF16)
    bjoint = din("bjoint", (128, 8), F32)
    wgate = [din(f"wgate{g}", (128, 4, 16, 256), BF16) for g in range(3)]
    bgate = [din(f"bgate{g}", (128, 8), F32) for g in range(3)]
    wo1 = din("wo1", (128, 8, 48, 256), BF16)
    bo1 = din("bo1", (128, 16), F32)
    wo2 = din("wo2", (128, 4, 16, 256), BF16)
    bo2 = din("bo2", (128, 8), F32)
    lng = [din(f"lng{i}", (128, 8), F32) for i in range(3)]
    lnb = [din(f"lnb{i}", (128, 8), F32) for i in range(3)]
    sel_d = din("sel_c", (8, 8 * 128), BF16)
    o32_d = din("o32_c", (128, 64), BF16)
    i128_d = din("i128_c", (128, 128), BF16)
    y_d = nc.dram_tensor("y", [128, 8, BC], BF16, kind="ExternalOutput").ap()

    with tile.TileContext(nc) as tc, ExitStack() as ctx:
        P = lambda **kw: ctx.enter_context(tc.tile_pool(**kw))
        cst = P(name="cst", bufs=1)
        wgp = P(name="wgp", bufs=2)                 # [128,8,256] weight blocks
        mmp = P(name="mmp", bufs=4, space="PSUM")   # 4 x [128,512] banks
        scp = P(name="scp", bufs=1, space="PSUM")   # [8,4,N] = 2 banks
        brp = P(name="brp", bufs=2, space="PSUM")   # 2 x [128,N] banks
        expp = P(name="expp", bufs=2)   # xp [128,8,4,N]; also h1
        enhp = P(name="enhp", bufs=2)   # t_enh, s_enh [128,8,4,N]
        qkvp = P(name="qkvp", bufs=2)   # k4, v4 [128,4,8,N]
        acc4p = P(name="acc4p", bufs=1)  # merged q/acc + jacc [128,4,8,N]
        actp = P(name="actp", bufs=3)   # sum_t,sum_s,joint,vj,qj [128,8,N]
        meanp = P(name="meanp", bufs=5)  # mts,mst,mtj,mjt,mjs
        accp = P(name="accp", bufs=2)   # cacc/f2/h2 [128,8,N]
        gatep = P(name="gatep", bufs=2)
        qkp = P(name="qkp", bufs=2)     # qk products, sq, av tmp [128,8,N]
        xbp = P(name="xbp", bufs=2)     # xt_b, xs_b (live whole pass)
        xinp = P(name="xinp", bufs=1)   # [128,4,N] f32 staging
        smp = P(name="smp", bufs=1)     # softmax exp [8,4,N] bf16
        smdp = P(name="smdp", bufs=2)   # softmax denom [8,N] f32
        smbp = P(name="smbp", bufs=4)   # a_l bf16 [8,N]
        smrp = P(name="smrp", bufs=3)   # LN scalars [1,N] f32
        smabp = P(name="smabp", bufs=4)  # abar f32 [8,N]

        _tc = [0]
        def T(pool, shape, dtype, tag):
            _tc[0] += 1
            return pool.tile(shape, dtype, tag=tag, name=f"{tag}_{_tc[0]}")

        ones_b = T(cst, [128, 1], BF16, "ones_b")
        nc.any.memset(ones_b[:], 1.0)
        onerow_f = T(cst, [1, 128], F32, "onerow_f")
        nc.any.memset(onerow_f[:], 1.0)
        sel = T(cst, [8, 8 * 128], BF16, "sel")
        nc.sync.dma_start(out=sel[:], in_=sel_d)
        o32 = T(cst, [128, 64], BF16, "o32")   # all-ones column at col 32
        nc.sync.dma_start(out=o32[:], in_=o32_d)
        i128 = T(cst, [128, 128], BF16, "i128")
        nc.sync.dma_start(out=i128[:], in_=i128_d)
        eps_t = T(cst, [1, 1], F32, "eps_t")
        nc.any.memset(eps_t[:], EPS)

        def ctile(name, ap):
            t = cst.tile(list(ap.shape), ap.dtype, tag=name)
            nc.sync.dma_start(out=t[:], in_=ap)
            return t

        bexp_t = [ctile(f"bexp{j}", bexp[j]) for j in range(2)]
        bqkv_t = [ctile(f"bqkv{i}", bqkv[i]) for i in range(5)]
        bout_t = [ctile(f"bout{i}", bout[i]) for i in range(5)]
        bjoint_t = ctile("bjoint", bjoint)
        bgate_t = [ctile(f"bgate{g}", bgate[g]) for g in range(3)]
        bo1_t = ctile("bo1", bo1)
        bo2_t = ctile("bo2", bo2)
        lng_t = [ctile(f"lng{i}", lng[i]) for i in range(3)]
        lnb_t = [ctile(f"lnb{i}", lnb[i]) for i in range(3)]

        def projS(w_d, M, src_pair, evict2, wcol0=0, npair=2, extra_mm=None):
            """Shared-weight projection, K=1024. src_pair(kt, p) -> [128,2,N]
            moving pair; two pairs (4 sources) per stationary load.
            evict2(mj, psums): psums[p] = [128,512] = pair p's two outputs.
            extra_mm(mj, p, psum): optional accumulate hook (residual)."""
            nmt = M // 128
            for mj0 in range(0, nmt, 2):
                gm = min(2, nmt - mj0)
                wt = T(wgp, [128, 8, 256], BF16, "wg")
                nc.sync.dma_start(
                    out=wt[:], in_=w_d[:, (wcol0 + mj0 * 128) // 256, :, :])
                for mj in range(mj0, mj0 + gm):
                    ps = [T(mmp, [128, 512], F32, "mm") for _ in range(npair)]
                    for kt in range(8):
                        w_sl = wt[:, kt, (mj - mj0) * 128 : (mj - mj0 + 1) * 128]
                        last = kt == 7 and extra_mm is None
                        for p in range(npair):
                            nc.tensor.matmul(ps[p][:], w_sl, src_pair(kt, p),
                                             start=(kt == 0), stop=last)
                    if extra_mm is not None:
                        for p in range(npair):
                            extra_mm(mj, p, ps[p])
                    evict2(mj, ps)

        def projM(w_d, M, K, src, evict2m, wcol0=0):
            """Single-source projection. One PSUM bank per m-tile (interleaved
            accumulation groups must not share a bank: start=True clears the
            has-written bits bank-wide). evict2m(mj0, gm, ps_list)."""
            nmt, nkt = M // 128, K // 128
            for mj0 in range(0, nmt, 2):
                gm = min(2, nmt - mj0)
                ps = [T(mmp, [128, 512], F32, "mm") for _ in range(gm)]
                for kc0 in range(0, nkt, 8):
                    kc = min(8, nkt - kc0)
                    wt = T(wgp, [128, 8, 256], BF16, "wg")
                    nc.sync.dma_start(
                        out=wt[:, :kc, :],
                        in_=w_d[:, (wcol0 + mj0 * 128) // 256,
                                kc0 : kc0 + kc, :])
                    for kt in range(kc0, kc0 + kc):
                        s = src(kt)
                        for mi in range(gm):
                            nc.tensor.matmul(
                                ps[mi][:, 0:256],
                                wt[:, kt - kc0, mi * 128 : (mi + 1) * 128], s,
                                start=(kt == 0), stop=(kt == nkt - 1))
                evict2m(mj0, gm, ps)

        def ev_split(dsts_of_mj, btile, bcol_of_mj, func=AF.Identity):
            """projM eviction: per-mtile ACT evicts [128,256] with bias."""
            def _ev(mj0, gm, ps):
                for mi in range(gm):
                    nc.scalar.activation(
                        dsts_of_mj(mj0 + mi), ps[mi][:, 0:256],
                        func, bias=btile[:, bcol_of_mj(mj0 + mi)
                                         : bcol_of_mj(mj0 + mi) + 1])
            return _ev

        def scores_all(q_sl, k4):
            """psum [8,4,N]: row h of col-block e2 = q[h].k[e2,h] (q pre-scaled).
            q_sl [128,8,N] contiguous; k4 [128,4,8,N] e-major."""
            sp = T(scp, [8, 4, N], F32, "sc")
            for e2 in range(4):
                p = T(qkp, [128, 8, N], BF16, "qk")
                nc.vector.tensor_tensor(
                    out=p[:], in0=q_sl, in1=k4[:, e2, :, :], op=ALU.mult)
                for kt in range(8):
                    nc.tensor.matmul(sp[:, e2, :], o32[:, 32 - kt : 40 - kt],
                                     p[:, kt, :], start=(kt == 0), stop=(kt == 7))
            return sp

        def softmax_tiles(sp):
            """sp [8,4,N] psum scores -> 4 bf16 [8,N] attention-weight tiles."""
            et = T(smp, [8, 4, N], BF16, "sm")
            nc.scalar.activation(et[:], sp[:], AF.Exp)
            d = T(smdp, [8, N], F32, "smd")
            nc.vector.tensor_add(out=d[:], in0=et[:, 0, :], in1=et[:, 1, :])
            for e2 in (2, 3):
                nc.vector.tensor_add(out=d[:], in0=d[:], in1=et[:, e2, :])
            r = T(smdp, [8, N], F32, "smd")
            nc.vector.reciprocal_approx_fast(out=r[:], in_=d[:])
            outs = []
            for e2 in range(4):
                a = T(smbp, [8, N], BF16, "smb")
                nc.vector.tensor_tensor(out=a[:], in0=et[:, e2, :], in1=r[:],
                                        op=ALU.mult)
                outs.append(a)
            return outs

        def av_accum(a_list, v4, dst_sl):
            """dst_sl [128,8,N] contiguous = sum_e2 bcast(a_list[e2]) * V[e2].
            v4 [128,4,8,N] e-major. Adds run on GpSimd (SBUF-only engine)."""
            for e2 in range(4):
                cur = dst_sl if e2 == 0 else T(qkp, [128, 8, N], BF16, "qk")
                for mt in range(0, 8, 2):
                    bp = T(brp, [128, 2, N], F32, "br")
                    for q in range(2):
                        nc.tensor.matmul(
                            bp[:, q, :],
                            sel[:, (mt + q) * 128 : (mt + q + 1) * 128],
                            a_list[e2][:], start=True, stop=True)
                    nc.vector.tensor_tensor(
                        out=cur[:, mt : mt + 2, :], in0=bp[:],
                        in1=v4[:, e2, mt : mt + 2, :], op=ALU.mult)
                if e2 > 0:
                    nc.vector.tensor_add(out=dst_sl, in0=dst_sl, in1=cur[:])

        def ln_norm(x_sl, g_t, b_t, dst_of_mt, alt=0):
            """LayerNorm over the 1024 feats of x_sl [128,8,N] (bf16, in-place
            scratch); writes normalized*g+b to dst_of_mt(mt). alt=1 runs the
            per-mt chain on GpSimd instead of Vector."""
            sq = T(qkp, [128, 8, N], BF16, "qk")
            nc.vector.tensor_tensor(out=sq[:], in0=x_sl, in1=x_sl, op=ALU.mult)
            st_s = T(scp, [1, N], F32, "sc")
            for kt in range(8):
                nc.tensor.matmul(st_s[:], ones_b[:], x_sl[:, kt, :],
                                 start=(kt == 0), stop=(kt == 7))
            st_q = T(scp, [1, N], F32, "sc")
            for kt in range(8):
                nc.tensor.matmul(st_q[:], ones_b[:], sq[:, kt, :],
                                 start=(kt == 0), stop=(kt == 7))
            mean = T(smrp, [1, N], F32, "smr")
            nc.vector.tensor_scalar_mul(mean[:], st_s[:], 1.0 / HD)
            mb = T(brp, [128, N], F32, "br")
            nc.tensor.matmul(mb[:], onerow_f[:], mean[:], start=True, stop=True)
            msq = T(smrp, [1, N], F32, "smr")
            nc.vector.tensor_scalar_mul(msq[:], st_q[:], 1.0 / HD)
            var = T(smrp, [1, N], F32, "smr")
            nc.vector.tensor_tensor(out=var[:], in0=mean[:], in1=mean[:],
                                    op=ALU.mult)
            nc.vector.tensor_tensor(out=var[:], in0=msq[:], in1=var[:],
                                    op=ALU.subtract)
            std = T(smrp, [1, N], F32, "smr")
            nc.scalar.activation(std[:], var[:], AF.Sqrt, bias=eps_t[:])
            rstd = T(smrp, [1, N], F32, "smr")
            nc.vector.reciprocal_approx_fast(out=rstd[:], in_=std[:])
            rb = T(brp, [128, N], F32, "br")
            nc.tensor.matmul(rb[:], onerow_f[:], rstd[:], start=True, stop=True)
            # bf16 SBUF copies of the broadcasts: per-mt chain becomes
            # all-bf16/SBUF (2x DVE mode, GpSimd-eligible)
            mbb = T(qkp, [128, 2, N], BF16, "qk")
            nc.scalar.activation(mbb[:, 0, :], mb[:], AF.Identity, bias=0.0)
            nc.scalar.activation(mbb[:, 1, :], rb[:], AF.Identity, bias=0.0)
            eng = nc.vector
            for mt in range(8):
                eng.tensor_tensor(out=x_sl[:, mt, :], in0=x_sl[:, mt, :],
                                  in1=mbb[:, 0, :], op=ALU.subtract)
                eng.tensor_tensor(out=x_sl[:, mt, :], in0=x_sl[:, mt, :],
                                  in1=mbb[:, 1, :], op=ALU.mult)
                eng.tensor_scalar(
                    out=dst_of_mt(mt), in0=x_sl[:, mt, :],
                    scalar1=g_t[:, mt : mt + 1], scalar2=b_t[:, mt : mt + 1],
                    op0=ALU.mult, op1=ALU.add)

        def kv_proj(mi, src4):
            """K/V projection of mha mi from src4 [128,8(kt),4(e),N] ->
            k4, v4 [128,4(e),8(mt),N] e-major."""
            k4 = T(qkvp, [128, 4, 8, N], BF16, "qkv")
            v4 = T(qkvp, [128, 4, 8, N], BF16, "qkv")
            def ev(mj, ps):
                dst = k4 if mj < 8 else v4
                bcol = 8 + mj      # k tiles: cols 8..15, v tiles: 16..23
                for p in range(2):
                    for q in range(2):
                        nc.scalar.activation(
                            dst[:, 2 * p + q, mj % 8, :],
                            ps[p][:, q * 256 : (q + 1) * 256],
                            AF.Identity,
                            bias=bqkv_t[mi][:, bcol : bcol + 1])
            projS(wqkv[mi], 2 * HD,
                  lambda kt, p: src4[:, kt, 2 * p : 2 * p + 2, :], ev,
                  wcol0=HD)
            return k4, v4

        def q_proj(mi, src4):
            """q projection into a merged q/acc tile: scores consume the e1
            slice, then av_accum overwrites it with the AV result in place."""
            qa = T(acc4p, [128, 4, 8, N], BF16, "acc4")
            def ev(mj, ps):
                for p in range(2):
                    for q in range(2):
                        nc.scalar.activation(
                            qa[:, 2 * p + q, mj, :],
                            ps[p][:, q * 256 : (q + 1) * 256],
                            AF.Identity, bias=bqkv_t[mi][:, mj : mj + 1])
            projS(wqkv[mi], HD,
                  lambda kt, p: src4[:, kt, 2 * p : 2 * p + 2, :], ev)
            return qa

        for c in range(NP):
            bs = slice(c * N, (c + 1) * N)

            def load_x(xd):
                xb = T(xbp, [128, 8, N], BF16, "xb")
                for h in range(2):
                    xf = T(xinp, [128, 4, N], F32, "xin")
                    nc.sync.dma_start(out=xf[:], in_=xd[:, 4 * h : 4 * h + 4, bs])
                    nc.vector.tensor_copy(out=xb[:, 4 * h : 4 * h + 4, :],
                                          in_=xf[:])
                return xb

            xt_b = load_x(xt_d)
            xs_b = load_x(xs_d)

            def expand(j, x_b):
                xp = T(expp, [128, 8, 4, N], BF16, "exp")
                # expand: m-tile m = e*8+mj -> xp[:, mj, e, :]
                def ev_exp(mj0, gm, ps):
                    for mi in range(gm):
                        m = mj0 + mi
                        nc.scalar.activation(
                            xp[:, m % 8, m // 8, :], ps[mi][:, 0:256],
                            AF.Identity, bias=bexp_t[j][:, m : m + 1])
                projM(wexp[j], E * HD, HD, lambda kt: x_b[:, kt, :], ev_exp)
                return xp

            def self_loop(qa, k4, v4, fillers={}):
                """self-attn e1 loop; av result replaces q in qa in place.
                fillers[e1] emits independent matmul work into the window."""
                for e1 in range(4):
                    a_l = softmax_tiles(scores_all(qa[:, e1, :, :], k4))
                    if e1 in fillers:
                        fillers[e1]()
                    av_accum(a_l, v4, qa[:, e1, :, :])

            def self_out(j, qa, xp, enh_dst):
                """out proj (+residual via identity matmul) -> enh_dst pre-LN."""
                def ex_res(mj, p, ps):
                    nc.tensor.matmul(ps[:], i128[:],
                                     xp[:, mj, 2 * p : 2 * p + 2, :],
                                     start=False, stop=True)
                def ev_out(mj, ps):
                    for p in range(2):
                        nc.scalar.activation(
                            enh_dst[:, mj, 2 * p : 2 * p + 2, :], ps[p][:],
                            AF.Identity, bias=bout_t[j][:, mj : mj + 1])
                projS(wout[j], HD,
                      lambda kt, p: qa[:, 2 * p : 2 * p + 2, kt, :],
                      ev_out, extra_mm=ex_res)

            def run_selfB(j, enh_dst, sum_dst):
                """LN each position of enh_dst in place; sum_dst = sum_e enh."""
                for e1 in range(4):
                    ln_norm(enh_dst[:, :, e1, :], lng_t[j], lnb_t[j],
                            lambda mt, e1=e1: enh_dst[:, mt, e1, :], alt=e1 % 2)
                t2 = T(qkp, [128, 8, N], BF16, "qk")
                nc.vector.tensor_add(out=sum_dst[:], in0=enh_dst[:, :, 0, :],
                                     in1=enh_dst[:, :, 1, :])
                nc.vector.tensor_add(out=t2[:], in0=enh_dst[:, :, 2, :],
                                     in1=enh_dst[:, :, 3, :])
                nc.vector.tensor_add(out=sum_dst[:], in0=sum_dst[:], in1=t2[:])

            def cross_loop(qa, k4, fillers={}):
                """cross-attn e1 loop accumulating abar (mean attn weights)."""
                abar = [None] * 4
                for e1 in range(4):
                    a_l = softmax_tiles(scores_all(qa[:, e1, :, :], k4))
                    if e1 in fillers:
                        fillers[e1]()
                    for e2 in range(4):
                        if e1 == 0:
                            ab = T(smabp, [8, N], F32, "smab")
                            nc.vector.tensor_copy(out=ab[:], in_=a_l[e2][:])
                            abar[e2] = ab
                        else:
                            nc.vector.tensor_add(out=abar[e2][:], in0=abar[e2][:],
                                                 in1=a_l[e2][:])
                return abar

            def cross_fin(mi, abar, v4, dst):
                """abar-weighted AV + out proj (wout pre-scaled 0.25)."""
                abar_b = []
                for e2 in range(4):
                    ab = T(smbp, [8, N], BF16, "smb")
                    nc.vector.tensor_copy(out=ab[:], in_=abar[e2][:])
                    abar_b.append(ab)
                cacc = T(accp, [128, 8, N], BF16, "accx")
                av_accum(abar_b, v4, cacc[:])
                projM(wout[mi], HD, HD, lambda kt: cacc[:, kt, :],
                      ev_split(lambda mj: dst[:, mj, :], bout_t[mi],
                               lambda mj: mj))

            def run_gate(g, in_a, in_b):
                gt = T(gatep, [128, 8, N], BF16, "gate")
                projM(wgate[g], HD, 2 * HD,
                      lambda kt: in_a[:, kt, :] if kt < 8 else in_b[:, kt - 8, :],
                      ev_split(lambda mj: gt[:, mj, :], bgate_t[g],
                               lambda mj: mj, func=AF.Sigmoid))
                return gt

            t_enh = T(enhp, [128, 8, 4, N], BF16, "enh")
            s_enh = T(enhp, [128, 8, 4, N], BF16, "enh")
            sum_t = T(actp, [128, 8, N], BF16, "act")
            sum_s = T(actp, [128, 8, N], BF16, "act")

            # --- self-t, with self-s expand filling the first av window
            xp_t = expand(0, xt_b)
            k4t, v4t = kv_proj(0, xp_t)
            qa_t = q_proj(0, xp_t)
            box = {}
            self_loop(qa_t, k4t, v4t,
                      {0: lambda: box.__setitem__('xp_s', expand(1, xs_b)),
                       3: lambda: box.__setitem__('kv_s',
                                                  kv_proj(1, box['xp_s']))})
            xp_s = box['xp_s']
            k4s, v4s = box['kv_s']
            self_out(0, qa_t, xp_t, t_enh)
            qa_s = q_proj(1, xp_s)
            run_selfB(0, t_enh, sum_t)
            # --- self-s
            self_loop(qa_s, k4s, v4s)
            self_out(1, qa_s, xp_s, s_enh)
            qa_c2 = q_proj(2, t_enh)        # fills the LN(s) window
            run_selfB(1, s_enh, sum_s)

            joint = T(actp, [128, 8, N], BF16, "act")
            projM(wjoint, HD, 2 * HD,
                  lambda kt: sum_t[:, kt, :] if kt < 8 else sum_s[:, kt - 8, :],
                  ev_split(lambda mj: joint[:, mj, :], bjoint_t, lambda mj: mj))

            # --- cross ts / st, jx projections interleaved into av windows
            mts = T(meanp, [128, 8, N], BF16, "mean")
            mst = T(meanp, [128, 8, N], BF16, "mean")
            k4c2, v4c2 = kv_proj(2, s_enh)
            def fill_c2():
                box['kv_c3'] = kv_proj(3, t_enh)
                box['qa_c3'] = q_proj(3, s_enh)
            abar2 = cross_loop(qa_c2, k4c2, {3: fill_c2})
            k4c3, v4c3 = box['kv_c3']
            qa_c3 = box['qa_c3']
            cross_fin(2, abar2, v4c2, mts)
            # mha4 group fills the abar(c3)/cross_fin(3) window:
            # mtj == msj == Wout4 @ (Wv4 @ joint + bv4) + bout4
            mtj = T(meanp, [128, 8, N], BF16, "mean")
            def fill_c3():
                vj = T(actp, [128, 8, N], BF16, "act")
                projM(wqkv[4], HD, HD, lambda kt: joint[:, kt, :],
                      ev_split(lambda mj: vj[:, mj, :], bqkv_t[4],
                               lambda mj: 16 + mj), wcol0=2 * HD)
                projM(wout[4], HD, HD, lambda kt: vj[:, kt, :],
                      ev_split(lambda mj: mtj[:, mj, :], bout_t[4],
                               lambda mj: mj))
                qj = T(actp, [128, 8, N], BF16, "act")
                projM(wqkv[4], HD, HD, lambda kt: joint[:, kt, :],
                      ev_split(lambda mj: qj[:, mj, :], bqkv_t[4],
                               lambda mj: mj))
                box['qj'] = qj
            abar3 = cross_loop(qa_c3, k4c3, {3: fill_c3})
            qj = box['qj']
            cross_fin(3, abar3, v4c3, mst)

            # --- jx: single-query cross-attn (q = joint row)
            jacc = T(acc4p, [128, 4, 8, N], BF16, "acc4")
            k4j1, v4j1 = kv_proj(4, t_enh)
            a_l1 = softmax_tiles(scores_all(qj[:], k4j1))
            k4j2, v4j2 = kv_proj(4, s_enh)  # fills av(jx-t) window
            av_accum(a_l1, v4j1, jacc[:, 0, :, :])
            a_l2 = softmax_tiles(scores_all(qj[:], k4j2))
            gate_t = run_gate(0, mts, mtj)  # fills av(jx-s) window
            av_accum(a_l2, v4j2, jacc[:, 1, :, :])
            mjt = T(meanp, [128, 8, N], BF16, "mean")
            mjs = T(meanp, [128, 8, N], BF16, "mean")
            def ev_jx(mj, ps):
                for jj, dst in enumerate((mjt, mjs)):
                    nc.scalar.activation(
                        dst[:, mj, :], ps[0][:, jj * 256 : (jj + 1) * 256],
                        AF.Identity, bias=bout_t[4][:, mj : mj + 1])
            projS(wout[4], HD, lambda kt, p: jacc[:, 0:2, kt, :], ev_jx,
                  npair=1)

            # gates (sigmoid) + fused mults, interleaved to bound liveness
            f2 = T(accp, [128, 8, N], BF16, "accx")
            nc.vector.tensor_tensor(out=f2[:], in0=gate_t[:], in1=mtj[:],
                                    op=ALU.mult)
            nc.vector.tensor_tensor(out=mts[:], in0=gate_t[:], in1=mts[:],
                                    op=ALU.mult)
            gate_s = run_gate(1, mst, mtj)
            nc.vector.tensor_tensor(out=mst[:], in0=gate_s[:], in1=mst[:],
                                    op=ALU.mult)
            nc.vector.tensor_tensor(out=mtj[:], in0=gate_s[:], in1=mtj[:],
                                    op=ALU.mult)
            gate_j = run_gate(2, mjt, mjs)
            nc.vector.tensor_tensor(out=mjt[:], in0=gate_j[:], in1=mjt[:],
                                    op=ALU.mult)
            nc.vector.tensor_tensor(out=mjs[:], in0=gate_j[:], in1=mjs[:],
                                    op=ALU.mult)
            fs = [mts, mst, f2, mtj, mjt, mjs]

            h1 = T(expp, [128, 8, 4, N], BF16, "exp")
            def ev_h1(mj0, gm, ps):
                for mi in range(gm):
                    m = mj0 + mi
                    nc.scalar.activation(
                        h1[:, m % 8, m // 8, :], ps[mi][:, 0:256],
                        AF.Relu, bias=bo1_t[:, m : m + 1])
            projM(wo1, 2 * HD, 6 * HD, lambda kt: fs[kt // 8][:, kt % 8, :],
                  ev_h1)
            h2 = T(accp, [128, 8, N], BF16, "accx")
            projM(wo2, HD, 2 * HD, lambda kt: h1[:, kt % 8, kt // 8, :],
                  ev_split(lambda mj: h2[:, mj, :], bo2_t, lambda mj: mj))

            # final LN (g,b pre-scaled by res_w) + (1-res_w)/2*(xt+xs)
            yt = T(qkp, [128, 8, N], BF16, "qk")
            ln_norm(h2[:], lng_t[2], lnb_t[2], lambda mt: yt[:, mt, :])
            cres = (1.0 - res_w) * 0.5
            orig = T(qkp, [128, 8, N], BF16, "qk")
            nc.vector.tensor_add(out=orig[:], in0=xt_b[:], in1=xs_b[:])
            nc.vector.tensor_scalar_mul(orig[:], orig[:], cres)
            nc.vector.tensor_add(out=yt[:], in0=yt[:], in1=orig[:])
            nc.sync.dma_start(out=y_d[:, :, bs], in_=yt[:])

    nc.compile()
    return nc


def _sel_const():
    s = np.zeros((8, 8 * 128), np.float32)
    for mt in range(8):
        s[mt, mt * 128 : (mt + 1) * 128] = 1.0
    return s.astype(BF)


def _o32_const():
    o = np.zeros((128, 64), np.float32)
    o[:, 32] = 1.0
    return o.astype(BF)


def _wl(w):
    """torch-style [M_out, K_in] -> pair-blocked [128, M/256, K/128, 256] bf16
    (one 2-mtile all-kt block contiguous per partition)."""
    a = np.asarray(w, np.float32).T          # [K, M]
    K, M = a.shape
    a = a.reshape(K // 128, 128, M // 256, 256).transpose(1, 2, 0, 3)
    return np.ascontiguousarray(a).astype(BF)


def _prep_inputs(i):
    res_w = float(np.asarray(i["res_w"]).reshape(-1)[0])
    sc = 1.0 / math.sqrt(128.0)

    shared = {
        "wexp0": _wl(i["exp_t_w"]), "wexp1": _wl(i["exp_s_w"]),
        "bexp0": _bias_cols(np.asarray(i["exp_t_b"]) + np.asarray(i["pos_enc"]).reshape(-1)),
        "bexp1": _bias_cols(np.asarray(i["exp_s_b"]) + np.asarray(i["pos_enc"]).reshape(-1)),
        "wjoint": _wl(np.asarray(i["joint_w"], np.float32) * 0.25),
        "bjoint": _bias_cols(i["joint_b"]),
        "wo1": _wl(i["out1_w"]), "bo1": _bias_cols(i["out1_b"]),
        "wo2": _wl(i["out2_w"]), "bo2": _bias_cols(i["out2_b"]),
        "sel_c": _sel_const(), "o32_c": _o32_const(),
        "i128_c": np.eye(128, dtype=np.float32).astype(BF),
    }
    for g in range(3):
        shared[f"wgate{g}"] = _wl(i["gate_w"][g])
        shared[f"bgate{g}"] = _bias_cols(i["gate_b"][g])
    for m in range(5):
        w = np.asarray(i["mha_in_w"][m], np.float32).copy()
        b = np.asarray(i["mha_in_b"][m], np.float32).copy()
        w[:HD] *= sc
        b[:HD] *= sc
        shared[f"wqkv{m}"] = _wl(w)
        shared[f"bqkv{m}"] = _bias_cols(b)
        wo = np.asarray(i["mha_out_w"][m], np.float32)
        if m in (2, 3):
            wo = wo * 0.25      # fold mean over the 4 query positions
        shared[f"wout{m}"] = _wl(wo)
        shared[f"bout{m}"] = _bias_cols(i["mha_out_b"][m])
    for ln in range(3):
        g = np.asarray(i["ln_g"][ln], np.float32)
        b = np.asarray(i["ln_b"][ln], np.float32)
        if ln == 2:
            g = g * res_w
            b = b * res_w
        shared[f"lng{ln}"] = _bias_cols(g)
        shared[f"lnb{ln}"] = _bias_cols(b)

    def shard_x(x, c):
        xc = np.asarray(x, np.float32)[c * BC : (c + 1) * BC, 0, :]  # [512,1024]
        return np.ascontiguousarray(xc.T.reshape(8, 128, BC).transpose(1, 0, 2))

    in_maps = []
    for c in range(NCORES):
        m = dict(shared)
        m["xt"] = shard_x(i["temporal_features"], c)
        m["xs"] = shard_x(i["spatial_features"], c)
        in_maps.append(m)
    return res_w, in_maps


def kernel(**inputs):
    res_w, in_maps = _prep_inputs(inputs)
    nc = build(res_w)
    res = bass_utils.run_bass_kernel_spmd(nc, in_maps, core_ids=list(range(NCORES)))
    outs = []
    for c in range(NCORES):
        y = res.results[c]["y"]                                   # [128,8,512]
        outs.append(np.asarray(y).astype(np.float32)
                    .transpose(1, 0, 2).reshape(HD, BC).T)
    return np.concatenate(outs, 0)[:, None, :].astype(np.float32)


# revision 40
# speedup vs baseline: 1.0448x; 1.0448x over previous
"""Trainium2 Bass kernel for nn_MISA (dense_transformer, data-parallel over 8 cores).

Layout: feature-major activations [feat_part=128, mtile, batch_cols] per core.
Batch 4096 -> 512 per core -> two passes of 256 columns.
All matmuls bf16 (fp32 PSUM accumulation); LN/softmax internals fp32.

v2 (DMA restructure): weights live in DRAM as [128, K/128, M] and each
weight tile is DMA'd exactly once per pass:
- projS shares one stationary weight load across the 4 expand positions
  (pairs of positions ride one matmul: moving [128,2,N] -> PSUM [128,512]).
- PSUM evictions run on the Scalar engine (Identity+bias) to unload DVE.
- self-attn residual is folded into the out-proj PSUM via an identity matmul,
  and the out-proj evicts in place over the expand tile.
- output is written bf16 and cast to f32 on host.

Structural simplifications (exact, not approximations):
- attention with all-equal keys/values (q/k/v = broadcast joint row) is the
  identity on v: cross_tj == cross_sj == out_proj4(v_proj4(joint)).
- mean over query positions commutes with out_proj and with A@V, so the six
  cross outputs never materialize per-query outputs (abar-weighted V only).
- all-equal queries (j as q): single query row, output equals its mean.
"""
import sys, math
from contextlib import ExitStack
sys.path.insert(0, "/opt/trn_rl_repo")

import numpy as np
import ml_dtypes

import concourse.bass as bass
import concourse.mybir as mybir
from concourse import bacc
import concourse.tile as tile
from concourse import bass_utils

F32 = mybir.dt.float32
BF16 = mybir.dt.bfloat16
AF = mybir.ActivationFunctionType
ALU = mybir.AluOpType
BF = ml_dtypes.bfloat16

H = 8
E = 4
HD = 1024
B = 4096
NCORES = 8
BC = B // NCORES          # 512 batch per core
NP = 2                    # passes per core
N = BC // NP              # 256 batch cols per pass
EPS = 1e-5


def _bias_cols(b):
    # [M] -> [128, M//128]: column m = per-partition bias of m-tile m
    return np.ascontiguousarray(np.asarray(b, np.float32).reshape(-1, 128).T)


def build(res_w: float):
    nc = bacc.Bacc("TRN2", target_bir_lowering=False, debug=False)

    def din(name, shape, dt):
        return nc.dram_tensor(name, list(shape), dt, kind="ExternalInput").ap()

    xt_d = din("xt", (128, 8, BC), F32)
    xs_d = din("xs", (128, 8, BC), F32)
    # weights pair-blocked: [128, M/256, K/128, 256] — one (2-mtile, all-kt)
    # block is contiguous per partition, so block DMAs run at full rate
    wexp = [din(f"wexp{j}", (128, 16, 8, 256), BF16) for j in range(2)]
    bexp = [din(f"bexp{j}", (128, 32), F32) for j in range(2)]
    wqkv = [din(f"wqkv{i}", (128, 12, 8, 256), BF16) for i in range(5)]
    bqkv = [din(f"bqkv{i}", (128, 24), F32) for i in range(5)]
    wout = [din(f"wout{i}", (128, 4, 8, 256), BF16) for i in range(5)]
    bout = [din(f"bout{i}", (128, 8), F32) for i in range(5)]
    wjoint = din("wjoint", (128, 4, 16, 256), BF16)
    bjoint = din("bjoint", (128, 8), F32)
    wgate = [din(f"wgate{g}", (128, 4, 16, 256), BF16) for g in range(3)]
    bgate = [din(f"bgate{g}", (128, 8), F32) for g in range(3)]
    wo1 = din("wo1", (128, 8, 48, 256), BF16)
    bo1 = din("bo1", (128, 16), F32)
    wo2 = din("wo2", (128, 4, 16, 256), BF16)
    bo2 = din("bo2", (128, 8), F32)
    lng = [din(f"lng{i}", (128, 8), F32) for i in range(3)]
    lnb = [din(f"lnb{i}", (128, 8), F32) for i in range(3)]
    sel_d = din("sel_c", (8, 8 * 128), BF16)
    o32_d = din("o32_c", (128, 64), BF16)
    i128_d = din("i128_c", (128, 128), BF16)
    y_d = nc.dram_tensor("y", [128, 8, BC], BF16, kind="ExternalOutput").ap()

    with tile.TileContext(nc) as tc, ExitStack() as ctx:
        P = lambda **kw: ctx.enter_context(tc.tile_pool(**kw))
        cst = P(name="cst", bufs=1)
        wgp = P(name="wgp", bufs=2)                 # [128,8,256] weight blocks
        mmp = P(name="mmp", bufs=4, space="PSUM")   # 4 x [128,512] banks
        scp = P(name="scp", bufs=1, space="PSUM")   # [8,4,N] = 2 banks
        brp = P(name="brp", bufs=2, space="PSUM")   # 2 x [128,N] banks
        expp = P(name="expp", bufs=2)   # xp [128,8,4,N]; also h1
        enhp = P(name="enhp", bufs=2)   # t_enh, s_enh [128,8,4,N]
        qkvp = P(name="qkvp", bufs=2)   # k4, v4 [128,4,8,N]
        acc4p = P(name="acc4p", bufs=1)  # merged q/acc + jacc [128,4,8,N]
        actp = P(name="actp", bufs=3)   # sum_t,sum_s,joint,vj,qj [128,8,N]
        meanp = P(name="meanp", bufs=5)  # mts,mst,mtj,mjt,mjs
        accp = P(name="accp", bufs=2)   # cacc/f2/h2 [128,8,N]
        gatep = P(name="gatep", bufs=2)
        qkp = P(name="qkp", bufs=2)     # qk products, sq, av tmp [128,8,N]
        xbp = P(name="xbp", bufs=2)     # xt_b, xs_b (live whole pass)
        xinp = P(name="xinp", bufs=1)   # [128,4,N] f32 staging
        smp = P(name="smp", bufs=1)     # softmax exp [8,4,N] bf16
        smdp = P(name="smdp", bufs=2)   # softmax denom [8,N] f32
        smbp = P(name="smbp", bufs=4)   # a_l bf16 [8,N]
        smrp = P(name="smrp", bufs=3)   # LN scalars [1,N] f32
        smabp = P(name="smabp", bufs=4)  # abar f32 [8,N]

        _tc = [0]
        def T(pool, shape, dtype, tag):
            _tc[0] += 1
            return pool.tile(shape, dtype, tag=tag, name=f"{tag}_{_tc[0]}")

        ones_b = T(cst, [128, 1], BF16, "ones_b")
        nc.any.memset(ones_b[:], 1.0)
        onerow_f = T(cst, [1, 128], F32, "onerow_f")
        nc.any.memset(onerow_f[:], 1.0)
        sel = T(cst, [8, 8 * 128], BF16, "sel")
        nc.sync.dma_start(out=sel[:], in_=sel_d)
        o32 = T(cst, [128, 64], BF16, "o32")   # all-ones column at col 32
        nc.sync.dma_start(out=o32[:], in_=o32_d)
        i128 = T(cst, [128, 128], BF16, "i128")
        nc.sync.dma_start(out=i128[:], in_=i128_d)
        eps_t = T(cst, [1, 1], F32, "eps_t")
        nc.any.memset(eps_t[:], EPS)

        def ctile(name, ap):
            t = cst.tile(list(ap.shape), ap.dtype, tag=name)
            nc.sync.dma_start(out=t[:], in_=ap)
            return t

        bexp_t = [ctile(f"bexp{j}", bexp[j]) for j in range(2)]
        bqkv_t = [ctile(f"bqkv{i}", bqkv[i]) for i in range(5)]
        bout_t = [ctile(f"bout{i}", bout[i]) for i in range(5)]
        bjoint_t = ctile("bjoint", bjoint)
        bgate_t = [ctile(f"bgate{g}", bgate[g]) for g in range(3)]
        bo1_t = ctile("bo1", bo1)
        bo2_t = ctile("bo2", bo2)
        lng_t = [ctile(f"lng{i}", lng[i]) for i in range(3)]
        lnb_t = [ctile(f"lnb{i}", lnb[i]) for i in range(3)]

        def projS(w_d, M, src_pair, evict2, wcol0=0, npair=2, extra_mm=None):
            """Shared-weight projection, K=1024. src_pair(kt, p) -> [128,2,N]
            moving pair; two pairs (4 sources) per stationary load.
            evict2(mj, psums): psums[p] = [128,512] = pair p's two outputs.
            extra_mm(mj, p, psum): optional accumulate hook (residual)."""
            nmt = M // 128
            for mj0 in range(0, nmt, 2):
                gm = min(2, nmt - mj0)
                wt = T(wgp, [128, 8, 256], BF16, "wg")
                nc.sync.dma_start(
                    out=wt[:], in_=w_d[:, (wcol0 + mj0 * 128) // 256, :, :])
                for mj in range(mj0, mj0 + gm):
                    ps = [T(mmp, [128, 512], F32, "mm") for _ in range(npair)]
                    for kt in range(8):
                        w_sl = wt[:, kt, (mj - mj0) * 128 : (mj - mj0 + 1) * 128]
                        last = kt == 7 and extra_mm is None
                        for p in range(npair):
                            nc.tensor.matmul(ps[p][:], w_sl, src_pair(kt, p),
                                             start=(kt == 0), stop=last)
                    if extra_mm is not None:
                        for p in range(npair):
                            extra_mm(mj, p, ps[p])
                    evict2(mj, ps)

        def projM(w_d, M, K, src, evict2m, wcol0=0):
            """Single-source projection. One PSUM bank per m-tile (interleaved
            accumulation groups must not share a bank: start=True clears the
            has-written bits bank-wide). evict2m(mj0, gm, ps_list)."""
            nmt, nkt = M // 128, K // 128
            for mj0 in range(0, nmt, 2):
                gm = min(2, nmt - mj0)
                ps = [T(mmp, [128, 512], F32, "mm") for _ in range(gm)]
                for kc0 in range(0, nkt, 8):
                    kc = min(8, nkt - kc0)
                    wt = T(wgp, [128, 8, 256], BF16, "wg")
                    nc.sync.dma_start(
                        out=wt[:, :kc, :],
                        in_=w_d[:, (wcol0 + mj0 * 128) // 256,
                                kc0 : kc0 + kc, :])
                    for kt in range(kc0, kc0 + kc):
                        s = src(kt)
                        for mi in range(gm):
                            nc.tensor.matmul(
                                ps[mi][:, 0:256],
                                wt[:, kt - kc0, mi * 128 : (mi + 1) * 128], s,
                                start=(kt == 0), stop=(kt == nkt - 1))
                evict2m(mj0, gm, ps)

        def ev_split(dsts_of_mj, btile, bcol_of_mj, func=AF.Identity):
            """projM eviction: per-mtile ACT evicts [128,256] with bias."""
            def _ev(mj0, gm, ps):
                for mi in range(gm):
                    nc.scalar.activation(
                        dsts_of_mj(mj0 + mi), ps[mi][:, 0:256],
                        func, bias=btile[:, bcol_of_mj(mj0 + mi)
                                         : bcol_of_mj(mj0 + mi) + 1])
            return _ev

        def scores_all(q_sl, k4):
            """psum [8,4,N]: row h of col-block e2 = q[h].k[e2,h] (q pre-scaled).
            q_sl [128,8,N] contiguous; k4 [128,4,8,N] e-major."""
            sp = T(scp, [8, 4, N], F32, "sc")
            for e2 in range(4):
                p = T(qkp, [128, 8, N], BF16, "qk")
                nc.vector.tensor_tensor(
                    out=p[:], in0=q_sl, in1=k4[:, e2, :, :], op=ALU.mult)
                for kt in range(8):
                    nc.tensor.matmul(sp[:, e2, :], o32[:, 32 - kt : 40 - kt],
                                     p[:, kt, :], start=(kt == 0), stop=(kt == 7))
            return sp

        def softmax_tiles(sp):
            """sp [8,4,N] psum scores -> 4 bf16 [8,N] attention-weight tiles."""
            et = T(smp, [8, 4, N], BF16, "sm")
            nc.scalar.activation(et[:], sp[:], AF.Exp)
            d = T(smdp, [8, N], F32, "smd")
            nc.vector.tensor_add(out=d[:], in0=et[:, 0, :], in1=et[:, 1, :])
            for e2 in (2, 3):
                nc.vector.tensor_add(out=d[:], in0=d[:], in1=et[:, e2, :])
            r = T(smdp, [8, N], F32, "smd")
            nc.vector.reciprocal_approx_fast(out=r[:], in_=d[:])
            outs = []
            for e2 in range(4):
                a = T(smbp, [8, N], BF16, "smb")
                nc.vector.tensor_tensor(out=a[:], in0=et[:, e2, :], in1=r[:],
                                        op=ALU.mult)
                outs.append(a)
            return outs

        def av_accum(a_list, v4, dst_sl):
            """dst_sl [128,8,N] contiguous = sum_e2 bcast(a_list[e2]) * V[e2].
            v4 [128,4,8,N] e-major. Adds run on GpSimd (SBUF-only engine)."""
            for e2 in range(4):
                cur = dst_sl if e2 == 0 else T(qkp, [128, 8, N], BF16, "qk")
                for mt in range(0, 8, 2):
                    bp = T(brp, [128, 2, N], F32, "br")
                    for q in range(2):
                        nc.tensor.matmul(
                            bp[:, q, :],
                            sel[:, (mt + q) * 128 : (mt + q + 1) * 128],
                            a_list[e2][:], start=True, stop=True)
                    nc.vector.tensor_tensor(
                        out=cur[:, mt : mt + 2, :], in0=bp[:],
                        in1=v4[:, e2, mt : mt + 2, :], op=ALU.mult)
                if e2 > 0:
                    nc.vector.tensor_add(out=dst_sl, in0=dst_sl, in1=cur[:])

        def ln_norm(x_sl, g_t, b_t, dst_of_mt, alt=0):
            """LayerNorm over the 1024 feats of x_sl [128,8,N] (bf16, in-place
            scratch); writes normalized*g+b to dst_of_mt(mt). alt=1 runs the
            per-mt chain on GpSimd instead of Vector."""
            sq = T(qkp, [128, 8, N], BF16, "qk")
            nc.vector.tensor_tensor(out=sq[:], in0=x_sl, in1=x_sl, op=ALU.mult)
            st_s = T(scp, [1, N], F32, "sc")
            for kt in range(8):
                nc.tensor.matmul(st_s[:], ones_b[:], x_sl[:, kt, :],
                                 start=(kt == 0), stop=(kt == 7))
            st_q = T(scp, [1, N], F32, "sc")
            for kt in range(8):
                nc.tensor.matmul(st_q[:], ones_b[:], sq[:, kt, :],
                                 start=(kt == 0), stop=(kt == 7))
            mean = T(smrp, [1, N], F32, "smr")
            nc.vector.tensor_scalar_mul(mean[:], st_s[:], 1.0 / HD)
            mb = T(brp, [128, N], F32, "br")
            nc.tensor.matmul(mb[:], onerow_f[:], mean[:], start=True, stop=True)
            msq = T(smrp, [1, N], F32, "smr")
            nc.vector.tensor_scalar_mul(msq[:], st_q[:], 1.0 / HD)
            var = T(smrp, [1, N], F32, "smr")
            nc.vector.tensor_tensor(out=var[:], in0=mean[:], in1=mean[:],
                                    op=ALU.mult)
            nc.vector.tensor_tensor(out=var[:], in0=msq[:], in1=var[:],
                                    op=ALU.subtract)
            std = T(smrp, [1, N], F32, "smr")
            nc.scalar.activation(std[:], var[:], AF.Sqrt, bias=eps_t[:])
            rstd = T(smrp, [1, N], F32, "smr")
            nc.vector.reciprocal_approx_fast(out=rstd[:], in_=std[:])
            rb = T(brp, [128, N], F32, "br")
            nc.tensor.matmul(rb[:], onerow_f[:], rstd[:], start=True, stop=True)
            # bf16 SBUF copies of the broadcasts: per-mt chain becomes
            # all-bf16/SBUF (2x DVE mode, GpSimd-eligible)
            mbb = T(qkp, [128, 2, N], BF16, "qk")
            nc.scalar.activation(mbb[:, 0, :], mb[:], AF.Identity, bias=0.0)
            nc.scalar.activation(mbb[:, 1, :], rb[:], AF.Identity, bias=0.0)
            eng = nc.vector
            for mt in range(8):
                eng.tensor_tensor(out=x_sl[:, mt, :], in0=x_sl[:, mt, :],
                                  in1=mbb[:, 0, :], op=ALU.subtract)
                eng.tensor_tensor(out=x_sl[:, mt, :], in0=x_sl[:, mt, :],
                                  in1=mbb[:, 1, :], op=ALU.mult)
                eng.tensor_scalar(
                    out=dst_of_mt(mt), in0=x_sl[:, mt, :],
                    scalar1=g_t[:, mt : mt + 1], scalar2=b_t[:, mt : mt + 1],
                    op0=ALU.mult, op1=ALU.add)

        def kv_proj(mi, src4):
            """K/V projection of mha mi from src4 [128,8(kt),4(e),N] ->
            k4, v4 [128,4(e),8(mt),N] e-major."""
            k4 = T(qkvp, [128, 4, 8, N], BF16, "qkv")
            v4 = T(qkvp, [128, 4, 8, N], BF16, "qkv")
            def ev(mj, ps):
                dst = k4 if mj < 8 else v4
                bcol = 8 + mj      # k tiles: cols 8..15, v tiles: 16..23
                for p in range(2):
                    for q in range(2):
                        nc.scalar.activation(
                            dst[:, 2 * p + q, mj % 8, :],
                            ps[p][:, q * 256 : (q + 1) * 256],
                            AF.Identity,
                            bias=bqkv_t[mi][:, bcol : bcol + 1])
            projS(wqkv[mi], 2 * HD,
                  lambda kt, p: src4[:, kt, 2 * p : 2 * p + 2, :], ev,
                  wcol0=HD)
            return k4, v4

        def q_proj(mi, src4):
            """q projection into a merged q/acc tile: scores consume the e1
            slice, then av_accum overwrites it with the AV result in place."""
            qa = T(acc4p, [128, 4, 8, N], BF16, "acc4")
            def ev(mj, ps):
                for p in range(2):
                    for q in range(2):
                        nc.scalar.activation(
                            qa[:, 2 * p + q, mj, :],
                            ps[p][:, q * 256 : (q + 1) * 256],
                            AF.Identity, bias=bqkv_t[mi][:, mj : mj + 1])
            projS(wqkv[mi], HD,
                  lambda kt, p: src4[:, kt, 2 * p : 2 * p + 2, :], ev)
            return qa

        for c in range(NP):
            bs = slice(c * N, (c + 1) * N)

            def load_x(xd):
                xb = T(xbp, [128, 8, N], BF16, "xb")
                for h in range(2):
                    xf = T(xinp, [128, 4, N], F32, "xin")
                    nc.sync.dma_start(out=xf[:], in_=xd[:, 4 * h : 4 * h + 4, bs])
                    nc.vector.tensor_copy(out=xb[:, 4 * h : 4 * h + 4, :],
                                          in_=xf[:])
                return xb

            xt_b = load_x(xt_d)
            xs_b = load_x(xs_d)

            def expand(j, x_b):
                xp = T(expp, [128, 8, 4, N], BF16, "exp")
                # expand: m-tile m = e*8+mj -> xp[:, mj, e, :]
                def ev_exp(mj0, gm, ps):
                    for mi in range(gm):
                        m = mj0 + mi
                        nc.scalar.activation(
                            xp[:, m % 8, m // 8, :], ps[mi][:, 0:256],
                            AF.Identity, bias=bexp_t[j][:, m : m + 1])
                projM(wexp[j], E * HD, HD, lambda kt: x_b[:, kt, :], ev_exp)
                return xp

            def self_loop(qa, k4, v4, fillers={}):
                """self-attn e1 loop; av result replaces q in qa in place.
                fillers[e1] emits independent matmul work into the window."""
                for e1 in range(4):
                    a_l = softmax_tiles(scores_all(qa[:, e1, :, :], k4))
                    if e1 in fillers:
                        fillers[e1]()
                    av_accum(a_l, v4, qa[:, e1, :, :])

            def self_out(j, qa, xp, enh_dst):
                """out proj (+residual via identity matmul) -> enh_dst pre-LN."""
                def ex_res(mj, p, ps):
                    nc.tensor.matmul(ps[:], i128[:],
                                     xp[:, mj, 2 * p : 2 * p + 2, :],
                                     start=False, stop=True)
                def ev_out(mj, ps):
                    for p in range(2):
                        nc.scalar.activation(
                            enh_dst[:, mj, 2 * p : 2 * p + 2, :], ps[p][:],
                            AF.Identity, bias=bout_t[j][:, mj : mj + 1])
                projS(wout[j], HD,
                      lambda kt, p: qa[:, 2 * p : 2 * p + 2, kt, :],
                      ev_out, extra_mm=ex_res)

            def run_selfB(j, enh_dst, sum_dst):
                """LN each position of enh_dst in place; sum_dst = sum_e enh."""
                for e1 in range(4):
                    ln_norm(enh_dst[:, :, e1, :], lng_t[j], lnb_t[j],
                            lambda mt, e1=e1: enh_dst[:, mt, e1, :], alt=e1 % 2)
                t2 = T(qkp, [128, 8, N], BF16, "qk")
                nc.vector.tensor_add(out=sum_dst[:], in0=enh_dst[:, :, 0, :],
                                     in1=enh_dst[:, :, 1, :])
                nc.vector.tensor_add(out=t2[:], in0=enh_dst[:, :, 2, :],
                                     in1=enh_dst[:, :, 3, :])
                nc.vector.tensor_add(out=sum_dst[:], in0=sum_dst[:], in1=t2[:])

            def cross_loop(qa, k4, fillers={}):
                """cross-attn e1 loop accumulating abar (mean attn weights)."""
                abar = [None] * 4
                for e1 in range(4):
                    a_l = softmax_tiles(scores_all(qa[:, e1, :, :], k4))
                    if e1 in fillers:
                        fillers[e1]()
                    for e2 in range(4):
                        if e1 == 0:
                            ab = T(smabp, [8, N], F32, "smab")
                            nc.vector.tensor_copy(out=ab[:], in_=a_l[e2][:])
                            abar[e2] = ab
                        else:
                            nc.vector.tensor_add(out=abar[e2][:], in0=abar[e2][:],
                                                 in1=a_l[e2][:])
                return abar

            def cross_fin(mi, abar, v4, dst):
                """abar-weighted AV + out proj (wout pre-scaled 0.25)."""
                abar_b = []
                for e2 in range(4):
                    ab = T(smbp, [8, N], BF16, "smb")
                    nc.vector.tensor_copy(out=ab[:], in_=abar[e2][:])
                    abar_b.append(ab)
                cacc = T(accp, [128, 8, N], BF16, "accx")
                av_accum(abar_b, v4, cacc[:])
                projM(wout[mi], HD, HD, lambda kt: cacc[:, kt, :],
                      ev_split(lambda mj: dst[:, mj, :], bout_t[mi],
                               lambda mj: mj))

            def run_gate(g, in_a, in_b):
                gt = T(gatep, [128, 8, N], BF16, "gate")
                projM(wgate[g], HD, 2 * HD,
                      lambda kt: in_a[:, kt, :] if kt < 8 else in_b[:, kt - 8, :],
                      ev_split(lambda mj: gt[:, mj, :], bgate_t[g],
                               lambda mj: mj, func=AF.Sigmoid))
                return gt

            t_enh = T(enhp, [128, 8, 4, N], BF16, "enh")
            s_enh = T(enhp, [128, 8, 4, N], BF16, "enh")
            sum_t = T(actp, [128, 8, N], BF16, "act")
            sum_s = T(actp, [128, 8, N], BF16, "act")

            # --- self-t, with self-s expand filling the first av window
            xp_t = expand(0, xt_b)
            k4t, v4t = kv_proj(0, xp_t)
            qa_t = q_proj(0, xp_t)
            box = {}
            self_loop(qa_t, k4t, v4t,
                      {0: lambda: box.__setitem__('xp_s', expand(1, xs_b))})
            xp_s = box['xp_s']
            k4s, v4s = kv_proj(1, xp_s)     # fills the av(t) tail
            self_out(0, qa_t, xp_t, t_enh)
            qa_s = q_proj(1, xp_s)
            run_selfB(0, t_enh, sum_t)
            # --- self-s
            self_loop(qa_s, k4s, v4s)
            self_out(1, qa_s, xp_s, s_enh)
            qa_c2 = q_proj(2, t_enh)        # fills the LN(s) window
            run_selfB(1, s_enh, sum_s)

            joint = T(actp, [128, 8, N], BF16, "act")
            projM(wjoint, HD, 2 * HD,
                  lambda kt: sum_t[:, kt, :] if kt < 8 else sum_s[:, kt - 8, :],
                  ev_split(lambda mj: joint[:, mj, :], bjoint_t, lambda mj: mj))

            # --- cross ts / st, jx projections interleaved into av windows
            mts = T(meanp, [128, 8, N], BF16, "mean")
            mst = T(meanp, [128, 8, N], BF16, "mean")
            k4c2, v4c2 = kv_proj(2, s_enh)
            abar2 = cross_loop(qa_c2, k4c2)
            k4c3, v4c3 = kv_proj(3, t_enh)  # fills cross_fin(2) window
            qa_c3 = q_proj(3, s_enh)
            cross_fin(2, abar2, v4c2, mts)
            abar3 = cross_loop(qa_c3, k4c3)
            # mha4 group fills the cross_fin(3) window:
            # mtj == msj == Wout4 @ (Wv4 @ joint + bv4) + bout4
            vj = T(actp, [128, 8, N], BF16, "act")
            projM(wqkv[4], HD, HD, lambda kt: joint[:, kt, :],
                  ev_split(lambda mj: vj[:, mj, :], bqkv_t[4],
                           lambda mj: 16 + mj), wcol0=2 * HD)
            mtj = T(meanp, [128, 8, N], BF16, "mean")
            projM(wout[4], HD, HD, lambda kt: vj[:, kt, :],
                  ev_split(lambda mj: mtj[:, mj, :], bout_t[4], lambda mj: mj))
            qj = T(actp, [128, 8, N], BF16, "act")
            projM(wqkv[4], HD, HD, lambda kt: joint[:, kt, :],
                  ev_split(lambda mj: qj[:, mj, :], bqkv_t[4], lambda mj: mj))
            cross_fin(3, abar3, v4c3, mst)

            # --- jx: single-query cross-attn (q = joint row)
            jacc = T(acc4p, [128, 4, 8, N], BF16, "acc4")
            k4j1, v4j1 = kv_proj(4, t_enh)
            a_l1 = softmax_tiles(scores_all(qj[:], k4j1))
            k4j2, v4j2 = kv_proj(4, s_enh)  # fills av(jx-t) window
            av_accum(a_l1, v4j1, jacc[:, 0, :, :])
            a_l2 = softmax_tiles(scores_all(qj[:], k4j2))
            gate_t = run_gate(0, mts, mtj)  # fills av(jx-s) window
            av_accum(a_l2, v4j2, jacc[:, 1, :, :])
            mjt = T(meanp, [128, 8, N], BF16, "mean")
            mjs = T(meanp, [128, 8, N], BF16, "mean")
            def ev_jx(mj, ps):
                for jj, dst in enumerate((mjt, mjs)):
                    nc.scalar.activation(
                        dst[:, mj, :], ps[0][:, jj * 256 : (jj + 1) * 256],
                        AF.Identity, bias=bout_t[4][:, mj : mj + 1])
            projS(wout[4], HD, lambda kt, p: jacc[:, 0:2, kt, :], ev_jx,
                  npair=1)

            # gates (sigmoid) + fused mults, interleaved to bound liveness
            f2 = T(accp, [128, 8, N], BF16, "accx")
            nc.vector.tensor_tensor(out=f2[:], in0=gate_t[:], in1=mtj[:],
                                    op=ALU.mult)
            nc.vector.tensor_tensor(out=mts[:], in0=gate_t[:], in1=mts[:],
                                    op=ALU.mult)
            gate_s = run_gate(1, mst, mtj)
            nc.vector.tensor_tensor(out=mst[:], in0=gate_s[:], in1=mst[:],
                                    op=ALU.mult)
            nc.vector.tensor_tensor(out=mtj[:], in0=gate_s[:], in1=mtj[:],
                                    op=ALU.mult)
            gate_j = run_gate(2, mjt, mjs)
            nc.vector.tensor_tensor(out=mjt[:], in0=gate_j[:], in1=mjt[:],
                                    op=ALU.mult)
            nc.vector.tensor_tensor(out=mjs[:], in0=gate_j[:], in1=mjs[:],
                                    op=ALU.mult)
            fs = [mts, mst, f2, mtj, mjt, mjs]

            h1 = T(expp, [128, 8, 4, N], BF16, "exp")
            def ev_h1(mj0, gm, ps):
                for mi in range(gm):
                    m = mj0 + mi
                    nc.scalar.activation(
                        h1[:, m % 8, m // 8, :], ps[mi][:, 0:256],
                        AF.Relu, bias=bo1_t[:, m : m + 1])
            projM(wo1, 2 * HD, 6 * HD, lambda kt: fs[kt // 8][:, kt % 8, :],
                  ev_h1)
            h2 = T(accp, [128, 8, N], BF16, "accx")
            projM(wo2, HD, 2 * HD, lambda kt: h1[:, kt % 8, kt // 8, :],
                  ev_split(lambda mj: h2[:, mj, :], bo2_t, lambda mj: mj))

            # final LN (g,b pre-scaled by res_w) + (1-res_w)/2*(xt+xs)
            yt = T(qkp, [128, 8, N], BF16, "qk")
            ln_norm(h2[:], lng_t[2], lnb_t[2], lambda mt: yt[:, mt, :])
            cres = (1.0 - res_w) * 0.5
            orig = T(qkp, [128, 8, N], BF16, "qk")
            nc.vector.tensor_add(out=orig[:], in0=xt_b[:], in1=xs_b[:])
            nc.vector.tensor_scalar_mul(orig[:], orig[:], cres)
            nc.vector.tensor_add(out=yt[:], in0=yt[:], in1=orig[:])
            nc.sync.dma_start(out=y_d[:, :, bs], in_=yt[:])

    nc.compile()
    return nc


def _sel_const():
    s = np.zeros((8, 8 * 128), np.float32)
    for mt in range(8):
        s[mt, mt * 128 : (mt + 1) * 128] = 1.0
    return s.astype(BF)


def _o32_const():
    o = np.zeros((128, 64), np.float32)
    o[:, 32] = 1.0
    return o.astype(BF)


def _wl(w):
    """torch-style [M_out, K_in] -> pair-blocked [128, M/256, K/128, 256] bf16
    (one 2-mtile all-kt block contiguous per partition)."""
    a = np.asarray(w, np.float32).T          # [K, M]
    K, M = a.shape
    a = a.reshape(K // 128, 128, M // 256, 256).transpose(1, 2, 0, 3)
    return np.ascontiguousarray(a).astype(BF)


def _prep_inputs(i):
    res_w = float(np.asarray(i["res_w"]).reshape(-1)[0])
    sc = 1.0 / math.sqrt(128.0)

    shared = {
        "wexp0": _wl(i["exp_t_w"]), "wexp1": _wl(i["exp_s_w"]),
        "bexp0": _bias_cols(np.asarray(i["exp_t_b"]) + np.asarray(i["pos_enc"]).reshape(-1)),
        "bexp1": _bias_cols(np.asarray(i["exp_s_b"]) + np.asarray(i["pos_enc"]).reshape(-1)),
        "wjoint": _wl(np.asarray(i["joint_w"], np.float32) * 0.25),
        "bjoint": _bias_cols(i["joint_b"]),
        "wo1": _wl(i["out1_w"]), "bo1": _bias_cols(i["out1_b"]),
        "wo2": _wl(i["out2_w"]), "bo2": _bias_cols(i["out2_b"]),
        "sel_c": _sel_const(), "o32_c": _o32_const(),
        "i128_c": np.eye(128, dtype=np.float32).astype(BF),
    }
    for g in range(3):
        shared[f"wgate{g}"] = _wl(i["gate_w"][g])
        shared[f"bgate{g}"] = _bias_cols(i["gate_b"][g])
    for m in range(5):
        w = np.asarray(i["mha_in_w"][m], np.float32).copy()
        b = np.asarray(i["mha_in_b"][m], np.float32).copy()
        w[:HD] *= sc
        b[:HD] *= sc
        shared[f"wqkv{m}"] = _wl(w)
        shared[f"bqkv{m}"] = _bias_cols(b)
        wo = np.asarray(i["mha_out_w"][m], np.float32)
        if m in (2, 3):
            wo = wo * 0.25      # fold mean over the 4 query positions
        shared[f"wout{m}"] = _wl(wo)
        shared[f"bout{m}"] = _bias_cols(i["mha_out_b"][m])
    for ln in range(3):
        g = np.asarray(i["ln_g"][ln], np.float32)
        b = np.asarray(i["ln_b"][ln], np.float32)
        if ln == 2:
            g = g * res_w
            b = b * res_w
        shared[f"lng{ln}"] = _bias_cols(g)
        shared[f"lnb{ln}"] = _bias_cols(b)

    def shard_x(x, c):
        xc = np.asarray(x, np.float32)[c * BC : (c + 1) * BC, 0, :]  # [512,1024]
        return np.ascontiguousarray(xc.T.reshape(8, 128, BC).transpose(1, 0, 2))

    in_maps = []
    for c in range(NCORES):
        m = dict(shared)
        m["xt"] = shard_x(i["temporal_features"], c)
        m["xs"] = shard_x(i["spatial_features"], c)
        in_maps.append(m)
    return res_w, in_maps


def kernel(**inputs):
    res_w, in_maps = _prep_inputs(inputs)
    nc = build(res_w)
    res = bass_utils.run_bass_kernel_spmd(nc, in_maps, core_ids=list(range(NCORES)))
    outs = []
    for c in range(NCORES):
        y = res.results[c]["y"]                                   # [128,8,512]
        outs.append(np.asarray(y).astype(np.float32)
                    .transpose(1, 0, 2).reshape(HD, BC).T)
    return np.concatenate(outs, 0)[:, None, :].astype(np.float32)
